# revision 1
# baseline (speedup 1.0000x reference)
"""Cross-attention Trainium2 kernel (8 NeuronCores, data-parallel).

Problem: B=4, C=64, H=64, W=64.
  q = conv1x1(v1, wq, bq); k = conv1x1(v2, wk, bk); v = conv1x1(v2, wv, bv)
  tokens n = (c, h) pairs (N = C*H = 4096), feature dim = W = 64
  out = softmax(q @ k^T) @ v

Sharding: core i handles batch b = i//2 and the q-token half h in
[32*(i%2), 32*(i%2+1)).  Every core needs the full v2[b] (k/v side) but only
its h-slice of v1[b] (q side).  No collectives.

Per-core algorithm:
  - scores computed TRANSPOSED: sT[j, i] = k_j . q_i with k-tokens j on
    partitions; after exp the tile is exactly the layout the P@V matmul
    streams (no attention-matrix transpose ever).
  - qT/kT held in FP16: a 32-bit moving operand streams at half rate
    through the PE, so fp32r scores matmuls cost 2x.  fp16 keeps 11
    mantissa bits (score error ~0.006 abs, irrelevant vs the bf16 P) and
    streams full rate.
  - q/k projections computed DIRECTLY in feature-major layout: x is DMA'd
    h-interleaved across the partition halves (h even -> partitions 0-63,
    odd -> 64-127); row-group-packed rank-64 matmuls with an [c, (2h, w)]
    x-slice as the stationary operand produce 256 tokens per matmul pair
    straight into PSUM (concurrent row-group matmuls MUST drain into
    different PSUM banks - same-bank is a fatal HW error).  This replaces
    the channel-major projection + 96 tiny PE transposes of the naive
    setup.  Biases (which ride the TOKEN index, token=(channel,h)) are
    folded in as broadcast adds during/after the PSUM->SBUF copies,
    using host-precomputed fp16 bias-tile patterns.  The q-token order
    is (hh2, hp, uu, o) (h = 4*uu + 2*hh2 + hp) so the Q copy is a
    single contiguous 2D fused bias-add per partition half; the output
    DMA mapping accounts for it (out h = 4*(2*tile+row_half) + pass).
  - a warm-up burst of dummy fp32 matmuls runs during the input DMAs so
    the HAM clock gate un-throttles the PE (1.2 -> 2.4 GHz) before the
    real compute starts, and the setup is kept dense so it stays warm.
  - no max subtraction (|s| <= ~60 here; exp fits fp32); softmax
    denominator via a ones-column appended to V.
  - main loop SOFTWARE-PIPELINED with lookahead 2; exp alternates between
    ScalarE (LUT exp) and VectorE (one-instruction Schraudolph bit-trick:
    int16(round(s*128*log2e + magic)) bitcast to bf16, ~3% per-element,
    mostly cancelled by softmax normalization; end-to-end ~5e-3).
  - V projection chunks are interleaved into pass 0's idle PE slots,
    borrowing scores PSUM tiles.
"""

import numpy as np

B, C, H, W = 4, 64, 64, 64
HH = H // 2            # h-rows per core (q-token half)
NQ = C * HH            # q tokens per core = 2048
NK = C * H             # k tokens = 4096
JB = NK // 128         # 32 j-blocks of 128 k-tokens
NP = JB // 2           # 16 row-packed j-block pairs
IP = 512               # i-span per pass (4 passes)
NCORES = 8

LOG2E = 1.4426950408889634
SCH_SCALE = 128.0 * LOG2E
SCH_BIAS = 16256.0 - 7.0   # centered so the sawtooth ratio has mean ~1
N_WARM = 22                # HAM warm-up matmuls

_CACHE = {}


def _build_nc():
    from contextlib import ExitStack

    import concourse.bass as bass
    import concourse.tile as tile
    from concourse import bacc, mybir
    from concourse.bass import ts
    from concourse.masks import make_identity

    F32 = mybir.dt.float32
    F32R = mybir.dt.float32r
    F16 = mybir.dt.float16
    BF16 = mybir.dt.bfloat16
    I16 = mybir.dt.int16
    AF = mybir.ActivationFunctionType
    ALU = mybir.AluOpType

    nc = bacc.Bacc(trn_type="TRN2", target_bir_lowering=False)

    x1_d = nc.declare_dram_parameter("x1", [C, HH, W], F32, False)
    x2_d = nc.declare_dram_parameter("x2", [C, H, W], F32, False)
    wq_d = nc.declare_dram_parameter("wq", [C, C], F32, False)
    wk_d = nc.declare_dram_parameter("wk", [C, C], F32, False)
    wv_d = nc.declare_dram_parameter("wv", [C, C], F32, False)
    bv_d = nc.declare_dram_parameter("bv", [1, C], F32, False)
    brdq_d = nc.declare_dram_parameter("brdq", [128, 512], mybir.dt.uint16, False)
    brdk_d = nc.declare_dram_parameter("brdk", [128, 512], mybir.dt.uint16, False)
    out_d = nc.declare_dram_parameter("out", [C, HH, W], F32, True)

    with ExitStack() as ctx:
        tc = ctx.enter_context(tile.TileContext(nc))
        cp = ctx.enter_context(tc.tile_pool(name="const", bufs=1))

        ident = cp.tile([128, 128], F32)
        make_identity(nc, ident[:, :])

        # prewarm the exp table set while input DMAs run
        warm = cp.tile([128, 2], F32)
        nc.vector.memset(warm[:, :], 0.0)
        nc.scalar.activation(warm[:, 0:1], warm[:, 1:2], AF.Exp)

        # h-interleaved x copies: h even -> partitions 0-63, odd -> 64-127
        x1_pk = cp.tile([128, HH // 2, W], F32R)
        x2_pk = cp.tile([128, H // 2, W], F32R)
        # channel-major x2 for the V projection (+ ones row for bias):
        # fp16 so the V matmuls stream full-rate; DMA to fp32 staging then
        # cast (DMA cannot convert)
        x2_st = cp.tile([C, H * W], F32)
        x2_sb = cp.tile([C + 1, H * W], F16)
        nc.gpsimd.memset(x2_sb[C : C + 1, :], 1.0)

        # vf_aug (128, JB, 65) bf16: partition p of block jb = v-token
        # (h = 2*jb + p//64, o = p%64); col 64 = 1.0 (denominator trick).
        vf = cp.tile([128, JB, 65], BF16)
        nc.gpsimd.memset(vf[:, :, :], 1.0)

        # brd_b[qk]: bias[o] tiled along the whole token axis, identical
        # on all w-partitions (host-precomputed fp16, doubled on-chip)
        brd_bq = cp.tile([128, NQ], F16)
        brd_bk = cp.tile([128, NQ], F16)

        # DMA queue order = criticality: x1 (Q path), weights/biases,
        # x2 h-interleaved (K path), x2 channel-major (V path, needed
        # deepest into pass 0)
        for h2 in range(2):
            nc.sync.dma_start(
                x1_pk[ts(h2, C), :, :],
                x1_d[:, :, :].rearrange("c (hh two) w -> c hh two w", two=2)[
                    :, :, h2, :
                ].bitcast(F32R),
            )
        w_sb = {}
        for name, wd in (("q", wq_d), ("k", wk_d), ("v", wv_d)):
            t = cp.tile([C, C], F32, tag=f"w_{name}")
            nc.sync.dma_start(t[:, :], wd[:, :])
            w_sb[name] = t
        wv_st = cp.tile([C + 1, C], F32, tag="wv_st")
        nc.sync.dma_start(wv_st[C : C + 1, :], bv_d[:, :])
        nc.sync.dma_start(brd_bq[:, 0:512], brdq_d[:, :].bitcast(F16))
        nc.sync.dma_start(brd_bk[:, 0:512], brdk_d[:, :].bitcast(F16))
        for h2 in range(2):
            nc.sync.dma_start(
                x2_pk[ts(h2, C), :, :],
                x2_d[:, :, :].rearrange("c (hh two) w -> c hh two w", two=2)[
                    :, :, h2, :
                ].bitcast(F32R),
            )
        for ch in range(2):
            nc.sync.dma_start(
                x2_st[:, ts(ch, H * W // 2)],
                x2_d[:, :, :].rearrange("c h w -> c (h w)")[
                    :, ts(ch, H * W // 2)
                ],
            )

        # wqT2/wkT2: [c, o] on both partition halves (rhs of the direct
        # projections); wTv: [c, o] + bias row (lhsT of the V projection)
        wqT2 = cp.tile([128, C], F32R)
        wkT2 = cp.tile([128, C], F32R)
        wTv = cp.tile([C + 1, C], F16)

        with tc.tile_pool(name="pp0", bufs=2, space="PSUM") as pp0:
            # HAM warm-up: dummy fp32 matmuls (quarter-rate => long busy
            # per instruction) while the DMAs stream in
            wps = pp0.tile([128, 128], F32, tag="warmmm")
            for _ in range(N_WARM):
                nc.tensor.matmul(wps[:, :], lhsT=ident[:, :], rhs=ident[:, :],
                                 start=True, stop=True)

            for name, dst in (("q", wqT2), ("k", wkT2)):
                ps = pp0.tile([C, C], F32, tag="wT_ps")
                nc.tensor.transpose(ps[:, :], w_sb[name][:, :], ident[0:C, 0:C])
                nc.vector.tensor_copy(dst[0:C, :], ps[:, :])
                nc.vector.tensor_copy(dst[C : 2 * C, :], ps[:, :])
            ps = pp0.tile([C, C], F32, tag="wT_ps")
            nc.tensor.transpose(ps[:, :], w_sb["v"][:, :], ident[0:C, 0:C])
            nc.vector.tensor_copy(wv_st[0:C, :], ps[:, :])
            nc.vector.tensor_copy(wTv[:, :], wv_st[:, :])
            for brd in (brd_bq, brd_bk):
                nc.vector.tensor_copy(brd[:, 512:1024], brd[:, 0:512])
                nc.vector.tensor_copy(brd[:, 1024:2048], brd[:, 0:1024])


        # ---- direct feature-major q/k projections (fp16 outputs) ----
        # qT2: (w, i=h*64+o) duplicated on both partition halves
        # kT2: (w, j) even j-blocks on partitions 0-63, odd on 64-127
        qT2 = cp.tile([128, NQ], F16)
        kT2 = cp.tile([128, NK // 2], F16)

        with tc.tile_pool(name="ppqk", bufs=2, space="PSUM") as ppqk:
            def qk_group(g, x_pk, wT2, is_q):
                # one group = 16 h's (tokens [1024g, 1024(g+1))).  The
                # stationary operand covers TWO adjacent h-pairs:
                # lhsT [c, (hh2, w)] -> psum partitions (hh2, w).  Eight
                # matmuls per group; h-parity hp lands in separate PSUM
                # banks: ps[64*hh2 + w, hp*512 + uu*64 + o]
                ps = ppqk.tile([128, 1024], F32, tag="qk")
                for uu in range(4):
                    hh0 = 8 * g + 2 * uu
                    for hp in range(2):
                        nc.tensor.matmul(
                            ps[:, hp * 512 + uu * C :][:, 0:C],
                            lhsT=x_pk[ts(hp, C), hh0 : hh0 + 2, :],
                            rhs=wT2[ts(hp, C), :],
                            start=True, stop=True,
                        )
                # psum (64*hh2 + w, hp*512 + uu*64 + o) ->
                #   h = 2*(8g + 2uu + hh2) + hp
                for hh2 in range(2):
                    src = ps[ts(hh2, C), :].rearrange(
                        "p (hp uu o) -> p uu hp o", hp=2, o=C
                    )
                    if is_q:
                        # token-in-group X = 4uu + 2hh2 + hp
                        dst = qT2[0:C, ts(g, 1024)].rearrange(
                            "p (uu hh2x hp o) -> p hh2x uu hp o",
                            uu=4, hh2x=2, hp=2,
                        )[:, hh2, :, :, :]
                    else:
                        # j-block jb = 8g + 2uu + hh2: parity hh2,
                        # pair p = 4g + uu
                        dst = kT2[64 * hh2 : 64 * hh2 + C, ts(g, 512)].rearrange(
                            "p (uu hp o) -> p uu hp o", uu=4, hp=2
                        )
                    eng = nc.vector if (hh2 == 0) else nc.scalar
                    if eng is nc.vector:
                        nc.vector.tensor_copy(dst, src[:, 0:4, :, :])
                    else:
                        nc.scalar.copy(dst, src[:, 0:4, :, :])

            # Q: one 32-h group; i-token order (hh2, hp, uu, o) chosen so
            # the PSUM->SBUF copy per hh2-half is a single 2D fused
            # bias-add (h = 4*uu + 2*hh2 + hp)
            psq = ppqk.tile([128, 1024], F32, tag="qk")
            for uu in range(8):
                for hp in range(2):
                    nc.tensor.matmul(
                        psq[:, hp * 512 + uu * C :][:, 0:C],
                        lhsT=x1_pk[ts(hp, C), 2 * uu : 2 * uu + 2, :],
                        rhs=wqT2[ts(hp, C), :],
                        start=True, stop=True,
                    )
            for hh2 in range(2):
                nc.vector.scalar_tensor_tensor(
                    qT2[0:C, ts(hh2, 1024)], psq[ts(hh2, C), :], 1.0,
                    brd_bq[0:C, 0:1024], ALU.mult, ALU.add,
                )
            nc.vector.tensor_copy(qT2[C : 2 * C, :], qT2[0:C, :])
            for g in range(NK // 1024):
                qk_group(g, x2_pk, wkT2, False)

            # biases ride the token index: one broadcast add per tensor
            # half (fp16 in-place)
            nc.vector.scalar_tensor_tensor(
                kT2[0:C, :], kT2[0:C, :], 1.0, brd_bk[0:C, :], ALU.mult, ALU.add
            )
            nc.vector.scalar_tensor_tensor(
                kT2[C : 2 * C, :], kT2[C : 2 * C, :], 1.0, brd_bk[C : 2 * C, :],
                ALU.mult, ALU.add,
            )

        # ---- main attention loop: 4 passes over i, row-packed j pairs ----
        LOOKAHEAD = 2
        outT_sb = cp.tile([C + 1, NQ], F32)
        with (
            tc.tile_pool(name="outp", bufs=1, space="PSUM") as op_pool,
            tc.tile_pool(name="sp", bufs=LOOKAHEAD + 1, space="PSUM") as sp,
            tc.tile_pool(name="ppool", bufs=4) as p_pool,
            tc.tile_pool(name="tp2", bufs=1, space="PSUM") as tp2,
            tc.tile_pool(name="opool", bufs=4) as o_pool,
            tc.tile_pool(name="rpool", bufs=4) as r_pool,
        ):
            outT_ps = None
            sps_ring = {}
            pt_ring = {}

            def emit_scores(ih, p):
                i0 = ih * IP
                sps = sp.tile([128, 2 * IP], F32, tag="scores")
                for blk in range(2):
                    half = 64 * blk
                    nc.tensor.matmul(
                        sps[:, ts(blk, IP)],
                        lhsT=kT2[half : half + 64, ts(p, 128)],
                        rhs=qT2[half : half + 64, i0 : i0 + IP],
                        start=True, stop=True,
                    )
                sps_ring[(ih, p)] = sps

            def emit_exp(ih, p):
                sps = sps_ring.pop((ih, p))
                pt = p_pool.tile([128, 2 * IP], BF16, tag="p")
                if p % 2 == 0:
                    nc.scalar.activation(pt[:, :], sps[:, :], AF.Exp)
                else:
                    # Schraudolph bit-trick exp on the DVE
                    nc.vector.tensor_scalar(
                        pt[:, :].bitcast(I16), sps[:, :], SCH_SCALE, SCH_BIAS,
                        ALU.mult, ALU.add,
                    )
                pt_ring[(ih, p)] = pt

            def emit_pv(ih, p):
                pt = pt_ring.pop((ih, p))
                for blk in range(2):
                    jb = 2 * p + blk
                    nc.tensor.matmul(
                        outT_ps[:, :],
                        lhsT=vf[:, jb, :],
                        rhs=pt[:, ts(blk, IP)],
                        start=(p == 0 and blk == 0),
                        stop=(p == NP - 1 and blk == 1),
                    )

            def emit_x2cast(ch):
                nc.vector.tensor_copy(
                    x2_sb[0:C, ts(ch, 1024)], x2_st[:, ts(ch, 1024)]
                )

            def emit_projv(ch):
                # V chunk ch (16 h's): borrows a scores PSUM tile; fills
                # vf blocks [8ch, 8ch+8)
                ps = sp.tile([128, 2 * IP], F32, tag="scores")
                for c2 in range(2):
                    nc.tensor.matmul(
                        ps[0:C, ts(c2, 512)],
                        lhsT=wTv[:, :],
                        rhs=x2_sb[:, ch * 1024 + c2 * 512 :][:, 0:512],
                        start=True, stop=True,
                    )
                pv = ps[0:C, :].rearrange("p (h2 h1 w) -> p h1 h2 w", h1=2, w=W)
                for h1 in range(2):
                    dst = vf[64 * h1 : 64 * (h1 + 1), ts(ch, 8), 0:W]
                    if h1 == 0:
                        nc.scalar.copy(dst, pv[:, h1, :, :])
                    else:
                        nc.vector.tensor_copy(dst, pv[:, h1, :, :])

            def emit_drain_head(ih, acc_ps):
                # copy pass ih's accumulator (with its denominator row) to
                # SBUF; per-tile normalization happens post-transpose where
                # the denominator is a per-partition scalar
                i0 = ih * IP
                nc.vector.tensor_copy(outT_sb[:, i0 : i0 + IP], acc_ps[:, :])

            def emit_drain_tile(ih, tt, pool=None):
                t = ih * (IP // 128) + tt
                ps = (pool or tp2).tile(
                    [128, C + 1], F32,
                    tag="ot" if pool is None else "scores",
                )
                nc.tensor.transpose(
                    ps[:, :], outT_sb[:, ts(t, 128)], ident[0 : C + 1, 0 : C + 1]
                )
                rec = r_pool.tile([128, 1], F32, tag="rec")
                nc.vector.reciprocal(rec[:, :], ps[:, C : C + 1])
                ot = o_pool.tile([128, C], F32, tag="o")
                nc.vector.tensor_scalar_mul(ot[:, :], ps[:, 0:C], rec[:, 0:1])
                # i = hh2*1024 + hp*512 + uu*64 + o; rows p = up*64 + o ->
                # out[o, h = 4*(2*tt+up) + ih, :]
                dest = out_d[:, :, :].rearrange(
                    "o (hb r) w -> o hb r w", r=4
                )[:, 2 * tt : 2 * tt + 2, ih, :].rearrange("o hb w -> hb o w")
                nc.sync.dma_start(dest, ot[:, :])

            NPASS = NQ // IP
            emit_x2cast(0)
            for ih in range(NPASS):
                prev_outT_ps = outT_ps
                emit_scores(ih, 0)
                if ih > 0:
                    emit_drain_head(ih - 1, prev_outT_ps)
                for p in range(1, LOOKAHEAD):
                    emit_scores(ih, p)
                outT_ps = op_pool.tile([C + 1, IP], F32, tag="outT")
                for p in range(NP):
                    if p + LOOKAHEAD < NP:
                        emit_scores(ih, p + LOOKAHEAD)
                    if ih == 0:
                        if p in (0, 3, 7):
                            emit_x2cast(p // 3 + 1)
                        if p in (0, 1, 5, 9):
                            emit_projv((p + 3) // 4)
                    if ih > 0 and p in (2, 5, 8, 11):
                        emit_drain_tile(ih - 1, (p - 2) // 3)
                    emit_exp(ih, p)
                    emit_pv(ih, p)
            emit_drain_head(NPASS - 1, outT_ps)
            for tt in range(IP // 128):
                # the scores pool is idle by now: borrow its buffers so the
                # final four transpose->normalize->DMA chains pipeline
                emit_drain_tile(NPASS - 1, tt, pool=sp)

    nc.compile()
    return nc


def _get_nc():
    if "nc" not in _CACHE:
        _CACHE["nc"] = _build_nc()
    return _CACHE["nc"]


def _in_maps(v1, v2, wq, bq, wk, bk, wv, bv):
    brdq = np.tile(
        np.asarray(bq, np.float32).astype(np.float16).view(np.uint16).reshape(1, C),
        (128, 8),
    )
    brdk = np.tile(
        np.asarray(bk, np.float32).astype(np.float16).view(np.uint16).reshape(1, C),
        (128, 8),
    )
    maps = []
    for core in range(NCORES):
        b, half = divmod(core, 2)
        maps.append({
            "x1": np.ascontiguousarray(
                v1[b, :, half * HH : (half + 1) * HH, :], dtype=np.float32
            ),
            "x2": np.ascontiguousarray(v2[b], dtype=np.float32),
            "wq": np.ascontiguousarray(wq, dtype=np.float32),
            "wk": np.ascontiguousarray(wk, dtype=np.float32),
            "wv": np.ascontiguousarray(wv, dtype=np.float32),
            "bv": np.ascontiguousarray(bv, dtype=np.float32).reshape(1, C),
            "brdq": brdq,
            "brdk": brdk,
        })
    return maps


def _gather(results, v1):
    out = np.zeros((B, C, H, W), dtype=np.float32)
    for core in range(NCORES):
        b, half = divmod(core, 2)
        out[b, :, half * HH : (half + 1) * HH, :] = results[core]["out"]
    return out


def _run(trace=False, **inputs):
    from concourse.bass_utils import run_bass_kernel_spmd

    nc = _get_nc()
    maps = _in_maps(**inputs)
    res = run_bass_kernel_spmd(
        nc, maps, core_ids=list(range(NCORES)), trace=trace
    )
    return _gather(res.results, inputs["v1"]), res


def kernel(**inputs):
    out, _ = _run(trace=False, **inputs)
    return out



# revision 5
# speedup vs baseline: 1.1012x; 1.1012x over previous
"""Cross-attention Trainium2 kernel (8 NeuronCores, data-parallel).

Problem: B=4, C=64, H=64, W=64.
  q = conv1x1(v1, wq, bq); k = conv1x1(v2, wk, bk); v = conv1x1(v2, wv, bv)
  tokens n = (c, h) pairs (N = C*H = 4096), feature dim = W = 64
  out = softmax(q @ k^T) @ v

Sharding: core i handles batch b = i//2 and the q-token half h in
[32*(i%2), 32*(i%2+1)).  Every core needs the full v2[b] (k/v side) but only
its h-slice of v1[b] (q side).  No collectives.

v2 structure (changes vs the 148us baseline, driven by its trace):
  - ALL inputs host-prepped in fp16 and pre-interleaved/pre-transposed:
    x1p/x2p h-parity-packed [hp*64+c, (hh w)], x2cm channel-major with a
    baked ones row (V path), weights pre-transposed (wqT/wkT doubled on both
    partition halves, wvT with a bias row), a DMA'd fp32 identity, and
    fp16 bias tile patterns.  Halves HBM traffic (2.9MB -> ~1.6MB), makes
    every DMA fully contiguous, and deletes the on-chip cast/transpose
    setup (the old GpSimd memset chain delayed the HAM warm-up to 10.8us
    and the PE ran pass 0 at HALF CLOCK for 24us).
  - warm-up burst depends only on a vector memset, so the PE is busy from
    ~4.7us and the HAM clock gate opens at ~8us, before the real compute.
  - Q/K projections stream fp16 at full PE rate; q/k biases (which ride
    the token axis) are added by GpSimd tensor_tensor on 512-col chunks,
    pipelined with the projection copies (critical path: qT2 chunk0 +
    kT2 group0 only).
  - main loop is a GLOBAL-slot software pipeline (64 slots = 4 passes x
    16 j-pairs): scores lookahead 2; exp alternates ScalarE LUT /
    VectorE Schraudolph; the two PV matmuls of a slot accumulate into
    TWO separate PSUM banks (psA even jb / psB odd jb) so they stream
    back-to-back with no same-bank RAW stall (+104ns/slot in the
    baseline trace).  Accumulator pairs double-buffer across passes
    (4 banks) so a pass's drain never gates the next pass's PV.
  - drains batched per pass in the FIRST slots of the next pass: copy
    psA (scalar) || copy psB (vector), vector add, 4 PE transposes into
    the just-freed accumulator banks, 4 reciprocals, 4 per-partition
    scaled copies (ScalarE activation Copy with per-partition scale),
    ONE output DMA per pass.  The baseline serialized all of this
    through the Vector FIFO and spent 9us after the last matmul.
"""

import numpy as np

B, C, H, W = 4, 64, 64, 64
HH = H // 2            # h-rows per core (q-token half)
NQ = C * HH            # q tokens per core = 2048
NK = C * H             # k tokens = 4096
JB = NK // 128         # 32 j-blocks of 128 k-tokens
NP = JB // 2           # 16 row-packed j-block pairs
IP = 512               # i-span per pass
NPASS = NQ // IP       # 4
NSLOT = NPASS * NP     # 64 global pipeline slots
NCORES = 8
LOOKAHEAD = 2

LOG2E = 1.4426950408889634
SCH_SCALE = 128.0 * LOG2E
SCH_BIAS = 16256.0 - 7.0   # centered so the sawtooth ratio has mean ~1
N_WARM = 18                # HAM warm-up matmuls (fp32, ~215ns each cold)

_CACHE = {}


def _build_nc():
    from contextlib import ExitStack

    import concourse.bass as bass
    import concourse.tile as tile
    from concourse import bacc, mybir
    from concourse.bass import ts

    F32 = mybir.dt.float32
    F16 = mybir.dt.float16
    BF16 = mybir.dt.bfloat16
    I16 = mybir.dt.int16
    AF = mybir.ActivationFunctionType
    ALU = mybir.AluOpType

    nc = bacc.Bacc(trn_type="TRN2", target_bir_lowering=False)

    x1p_d = nc.declare_dram_parameter("x1p", [128, (HH // 2) * W], F16, False)
    x2p_d = nc.declare_dram_parameter("x2p", [128, (H // 2) * W], F16, False)
    x2cm_d = nc.declare_dram_parameter("x2cm", [C + 1, H * W], F16, False)
    wqt2_d = nc.declare_dram_parameter("wqt2", [128, C], F16, False)
    wkt2_d = nc.declare_dram_parameter("wkt2", [128, C], F16, False)
    wtv_d = nc.declare_dram_parameter("wtv", [C + 1, C], F16, False)
    ident_d = nc.declare_dram_parameter("ident", [128, 128], F32, False)
    brdq_d = nc.declare_dram_parameter("brdq", [128, 512], F16, False)
    brdk_d = nc.declare_dram_parameter("brdk", [128, 512], F16, False)
    out_d = nc.declare_dram_parameter("out", [C, HH, W], F32, True)

    with ExitStack() as ctx:
        tc = ctx.enter_context(tile.TileContext(nc))
        cp = ctx.enter_context(tc.tile_pool(name="const", bufs=1))

        # prewarm the exp table set while input DMAs run
        warm = cp.tile([128, 2], F32)
        nc.vector.memset(warm[:, :], 0.0)
        nc.scalar.activation(warm[:, 0:1], warm[:, 1:2], AF.Exp)

        # warm-up feed tile: no gpsimd dependency, so the HAM burst starts
        # as soon as the engines boot
        wfill = cp.tile([128, 128], F32)
        nc.vector.memset(wfill[:, :], 0.0)

        x1p = cp.tile([128, (HH // 2) * W], F16)
        x2p = cp.tile([128, (H // 2) * W], F16)
        x2cm = cp.tile([C + 1, H * W], F16)
        wqt2 = cp.tile([128, C], F16)
        wkt2 = cp.tile([128, C], F16)
        wtv = cp.tile([C + 1, C], F16)
        ident = cp.tile([128, 128], F32)
        brdq = cp.tile([128, 512], F16)
        brdk = cp.tile([128, 512], F16)

        qT2 = cp.tile([128, NQ], F16)        # (w, i) doubled on both halves
        kT2 = cp.tile([128, NK // 2], F16)   # (w, j) even jb lower / odd upper
        vf = cp.tile([128, JB, 65], BF16)    # v-tokens on partitions, col64=1
        outT_A = cp.tile([C + 1, IP], F32)   # drained accumulators (SBUF)
        outT_B = cp.tile([C + 1, IP], F32)

        # DMA issue split across engine queues; most-critical first.
        nc.sync.dma_start(x1p[:, :], x1p_d[:, :])
        nc.sync.dma_start(wqt2[:, :], wqt2_d[:, :])
        nc.sync.dma_start(wkt2[:, :], wkt2_d[:, :])
        nc.sync.dma_start(wtv[:, :], wtv_d[:, :])
        nc.sync.dma_start(ident[:, :], ident_d[:, :])
        nc.gpsimd.dma_start(x2p[:, :], x2p_d[:, :])
        nc.gpsimd.dma_start(x2cm[:, :], x2cm_d[:, :])
        nc.gpsimd.dma_start(brdq[:, :], brdq_d[:, :])
        nc.gpsimd.dma_start(brdk[:, :], brdk_d[:, :])
        nc.gpsimd.memset(vf[:, :, 64:65], 1.0)

        # ---- HAM warm-up burst (PE busy from boot; ~4us of fp32 matmuls)
        with tc.tile_pool(name="ppw", bufs=1, space="PSUM") as ppw:
            wps = ppw.tile([128, 128], F32, tag="warm")
            for _ in range(N_WARM):
                nc.tensor.matmul(wps[:, :], lhsT=wfill[:, :], rhs=wfill[:, :],
                                 start=True, stop=True)

        # ---- projections: Q first (critical), then K groups, then V ----
        with (
            tc.tile_pool(name="ppq", bufs=1, space="PSUM") as ppq,
            tc.tile_pool(name="ppkv", bufs=3, space="PSUM") as ppkv,
        ):
            # --- emit helpers (copies/biases pipelined across engines:
            # scalar: qT2 chunks + kT2 upper halves + vf lower halves;
            # vector: kT2 lower halves + qT2 upper-half copies + vf upper;
            # gpsimd: per-512-col bias adds, pattern period 64) ---
            def k_mms(g):
                ps = ppkv.tile([128, 1024], F32, tag="kv")
                for uu in range(4):
                    for hp in range(2):
                        nc.tensor.matmul(
                            ps[:, hp * 512 + uu * C:][:, 0:C],
                            lhsT=x2p[ts(hp, C), 512 * g + 128 * uu:][:, 0:128],
                            rhs=wkt2[ts(hp, C), :],
                            start=True, stop=True,
                        )
                return ps

            def v_mms(ch):
                ps = ppkv.tile([128, 1024], F32, tag="kv")
                for c2 in range(2):
                    nc.tensor.matmul(
                        ps[0:C, ts(c2, 512)],
                        lhsT=wtv[:, :],
                        rhs=x2cm[:, ch * 1024 + c2 * 512:][:, 0:512],
                        start=True, stop=True,
                    )
                return ps

            def k_copies(g, ps):
                for hh2 in range(2):
                    src = ps[ts(hh2, C), :].rearrange(
                        "p (hp uu o) -> p uu hp o", hp=2, o=C
                    )
                    dst = kT2[64 * hh2: 64 * hh2 + C, ts(g, 512)].rearrange(
                        "p (uu hp o) -> p uu hp o", uu=4, hp=2
                    )
                    if hh2 == 0:
                        nc.vector.tensor_copy(dst, src[:, 0:4, :, :])
                    else:
                        nc.scalar.copy(dst, src[:, 0:4, :, :])
                nc.gpsimd.tensor_tensor(
                    kT2[:, ts(g, 512)], kT2[:, ts(g, 512)], brdk[:, :], ALU.add
                )

            def v_copies(ch, ps):
                pv = ps[0:C, :].rearrange(
                    "p (h2 h1 w) -> p h1 h2 w", h1=2, w=W
                )
                for h1 in range(2):
                    dst = vf[64 * h1: 64 * (h1 + 1), ts(ch, 8), 0:W]
                    if h1 == 0:
                        nc.scalar.copy(dst, pv[:, h1, :, :])
                    else:
                        nc.vector.tensor_copy(dst, pv[:, h1, :, :])

            def q_bias(ihc):
                nc.gpsimd.tensor_tensor(
                    qT2[0:C, ts(ihc, 512)], qT2[0:C, ts(ihc, 512)],
                    brdq[0:C, :], ALU.add,
                )
                nc.vector.tensor_copy(qT2[C:2 * C, ts(ihc, 512)],
                                      qT2[0:C, ts(ihc, 512)])

            # Q: token order i = hh2*1024 + hp*512 + uu*64 + o, h=4uu+2hh2+hp
            psq = ppq.tile([128, 1024], F32, tag="q")
            for uu in range(8):
                for hp in range(2):
                    nc.tensor.matmul(
                        psq[:, hp * 512 + uu * C:][:, 0:C],
                        lhsT=x1p[ts(hp, C), ts(uu, 128)],
                        rhs=wqt2[ts(hp, C), :],
                        start=True, stop=True,
                    )
            # K: jb = 8g + 2uu + hh2, scores pair p = 4g + uu
            kp0 = k_mms(0)
            kp1 = k_mms(1)
            nc.scalar.copy(qT2[0:C, 0:1024], psq[0:C, :])   # q chunk hh2=0
            k_copies(0, kp0)
            q_bias(0)
            kp2 = k_mms(2)
            q_bias(1)
            nc.scalar.copy(qT2[0:C, 1024:2048], psq[C:2 * C, :])
            k_copies(1, kp1)
            kp3 = k_mms(3)
            k_copies(2, kp2)
            vp0 = v_mms(0)
            k_copies(3, kp3)
            q_bias(2)
            q_bias(3)
            vp1 = v_mms(1)
            v_copies(0, vp0)
            vp2 = v_mms(2)
            v_copies(1, vp1)
            vp3 = v_mms(3)
            v_copies(2, vp2)
            v_copies(3, vp3)

        # ---- main attention loop: 64 global slots, psA/psB bank split ----
        with (
            tc.tile_pool(name="accp", bufs=1, space="PSUM") as accp,
            tc.tile_pool(name="sp", bufs=LOOKAHEAD, space="PSUM") as sp,
            tc.tile_pool(name="ppool", bufs=4) as p_pool,
            tc.tile_pool(name="opool", bufs=2) as o_pool,
            tc.tile_pool(name="rpool", bufs=2) as r_pool,
        ):
            # accumulator pairs: pass ih uses pair ih%2.  Full-bank [128,512]
            # tiles so the drain transposes can land in the freed banks.
            accA0 = accp.tile([128, IP], F32, tag="accA0", name="accA0")
            accB0 = accp.tile([128, IP], F32, tag="accB0", name="accB0")
            accA1 = accp.tile([128, IP], F32, tag="accA1", name="accA1")
            accB1 = accp.tile([128, IP], F32, tag="accB1", name="accB1")
            acc = [(accA0, accB0), (accA1, accB1)]
            sps_ring = {}
            pt_ring = {}
            o_ring = {}
            r_ring = {}

            def emit_scores(s):
                ih, p = divmod(s, NP)
                i0 = ih * IP
                sps = sp.tile([128, 2 * IP], F32, tag="scores")
                for blk in range(2):
                    half = 64 * blk
                    nc.tensor.matmul(
                        sps[:, ts(blk, IP)],
                        lhsT=kT2[half: half + 64, ts(p, 128)],
                        rhs=qT2[half: half + 64, i0: i0 + IP],
                        start=True, stop=True,
                    )
                sps_ring[s] = sps

            def emit_exp(s):
                sps = sps_ring.pop(s)
                pt = p_pool.tile([128, 2 * IP], BF16, tag="p")
                if s % 2 == 0:
                    nc.scalar.activation(pt[:, :], sps[:, :], AF.Exp)
                else:
                    # Schraudolph bit-trick exp on the DVE
                    nc.vector.tensor_scalar(
                        pt[:, :].bitcast(I16), sps[:, :], SCH_SCALE, SCH_BIAS,
                        ALU.mult, ALU.add,
                    )
                pt_ring[s] = pt

            def emit_pv(s):
                ih, p = divmod(s, NP)
                pt = pt_ring.pop(s)
                pA, pB = acc[ih % 2]
                for blk, dst in ((0, pA), (1, pB)):
                    nc.tensor.matmul(
                        dst[0:C + 1, :],
                        lhsT=vf[:, 2 * p + blk, :],
                        rhs=pt[:, ts(blk, IP)],
                        start=(p == 0), stop=(p == NP - 1),
                    )

            # drain steps for pass d (run during pass d+1 or post-loop)
            def emit_copy(d):
                pA, pB = acc[d % 2]
                nc.scalar.copy(outT_A[:, :], pA[0:C + 1, :])
                nc.vector.tensor_copy(outT_B[:, :], pB[0:C + 1, :])

            def emit_add(d):
                nc.vector.tensor_tensor(outT_A[:, :], outT_A[:, :],
                                        outT_B[:, :], ALU.add)

            # transpose t -> (tile, col0): t0/t2 -> A, t1/t3 -> B
            def t_slot(d, t):
                pA, pB = acc[d % 2]
                return (pA if t % 2 == 0 else pB), 65 * (t // 2)

            def emit_transposes(d):
                for t in range(4):
                    dst, c0 = t_slot(d, t)
                    nc.tensor.transpose(
                        dst[:, c0: c0 + 65], outT_A[:, ts(t, 128)],
                        ident[0:C + 1, 0:C + 1],
                    )

            def emit_recips(d):
                rec = r_pool.tile([128, 4], F32, tag="rec")
                r_ring[d] = rec
                for t in range(4):
                    src, c0 = t_slot(d, t)
                    nc.vector.reciprocal(rec[:, t: t + 1],
                                         src[:, c0 + 64: c0 + 65])

            def emit_muls(d):
                rec = r_ring.pop(d)
                ot = o_pool.tile([128, 4, W], F32, tag="o")
                o_ring[d] = ot
                for t in range(4):
                    src, c0 = t_slot(d, t)
                    nc.scalar.activation(
                        ot[:, t, :], src[:, c0: c0 + C], AF.Copy,
                        scale=rec[:, t: t + 1],
                    )

            def emit_outdma(d):
                ot = o_ring.pop(d)
                # partition p = up*64 + o; value out[o, h = 8t + 4up + d, w];
                # dest traversal (u, o, t, w) matches src (p=(u,o), t, w)
                dest = out_d[:, :, :].rearrange(
                    "o (t u r) w -> u o t r w", t=4, u=2, r=NPASS
                )[:, :, :, d, :]
                nc.sync.dma_start(dest, ot[:, :, :])

            emit_scores(0)
            emit_scores(1)
            for s in range(NSLOT):
                ih, p = divmod(s, NP)
                if s + LOOKAHEAD < NSLOT:
                    emit_scores(s + LOOKAHEAD)
                emit_exp(s)
                if ih > 0:
                    d = ih - 1
                    if p == 1:
                        emit_copy(d)
                    elif p == 2:
                        emit_add(d)
                    elif p == 4:
                        emit_transposes(d)
                    elif p == 5:
                        emit_recips(d)
                    elif p == 6:
                        emit_muls(d)
                    elif p == 7:
                        emit_outdma(d)
                emit_pv(s)

            d = NPASS - 1
            emit_copy(d)
            emit_add(d)
            emit_transposes(d)
            emit_recips(d)
            emit_muls(d)
            emit_outdma(d)

    nc.compile()
    return nc


def _get_nc():
    if "nc" not in _CACHE:
        _CACHE["nc"] = _build_nc()
    return _CACHE["nc"]


def _in_maps(v1, v2, wq, bq, wk, bk, wv, bv):
    f32, f16 = np.float32, np.float16
    wq = np.asarray(wq, f32)
    wk = np.asarray(wk, f32)
    wv = np.asarray(wv, f32)
    wqt2 = np.ascontiguousarray(np.tile(wq.T.astype(f16), (2, 1)))
    wkt2 = np.ascontiguousarray(np.tile(wk.T.astype(f16), (2, 1)))
    wtv = np.ascontiguousarray(
        np.concatenate([wv.T, np.asarray(bv, f32).reshape(1, C)], 0).astype(f16)
    )
    ident = np.eye(128, dtype=f32)
    brdq = np.ascontiguousarray(
        np.tile(np.asarray(bq, f32).astype(f16).reshape(1, C), (128, 8))
    )
    brdk = np.ascontiguousarray(
        np.tile(np.asarray(bk, f32).astype(f16).reshape(1, C), (128, 8))
    )
    maps = []
    for core in range(NCORES):
        b, half = divmod(core, 2)
        x1s = np.asarray(v1[b, :, half * HH: (half + 1) * HH, :], f32)
        x2s = np.asarray(v2[b], f32)
        x1p = np.ascontiguousarray(
            x1s.reshape(C, HH // 2, 2, W).transpose(2, 0, 1, 3)
            .reshape(128, (HH // 2) * W).astype(f16)
        )
        x2p = np.ascontiguousarray(
            x2s.reshape(C, H // 2, 2, W).transpose(2, 0, 1, 3)
            .reshape(128, (H // 2) * W).astype(f16)
        )
        x2cm = np.ascontiguousarray(
            np.concatenate([x2s.reshape(C, H * W),
                            np.ones((1, H * W), f32)], 0).astype(f16)
        )
        maps.append({
            "x1p": x1p, "x2p": x2p, "x2cm": x2cm,
            "wqt2": wqt2, "wkt2": wkt2, "wtv": wtv,
            "ident": ident, "brdq": brdq, "brdk": brdk,
        })
    return maps


def _gather(results, v1):
    out = np.zeros((B, C, H, W), dtype=np.float32)
    for core in range(NCORES):
        b, half = divmod(core, 2)
        out[b, :, half * HH: (half + 1) * HH, :] = results[core]["out"]
    return out


def _run(trace=False, **inputs):
    from concourse.bass_utils import run_bass_kernel_spmd

    nc = _get_nc()
    maps = _in_maps(**inputs)
    res = run_bass_kernel_spmd(
        nc, maps, core_ids=list(range(NCORES)), trace=trace
    )
    return _gather(res.results, inputs["v1"]), res


def kernel(**inputs):
    out, _ = _run(trace=False, **inputs)
    return out
